# revision 23
# baseline (speedup 1.0000x reference)
"""Masked 5x5 conv (PixelCNN 'A' mask) on 8 Trainium2 NeuronCores.

Problem (hardcoded): x[4,192,128,128] f32, weight[384,192,5,5] f32,
bias[384] f32, mask[4,1,128,128] i32.
out = where(window_any(mask), conv(x, weight*maskA) + bias, 0).

The 'A' causal mask keeps 12 of 25 taps: rows kh=0,1 fully, row kh=2 only
kw=0,1 -- i.e. every tap reads the current output row or rows above it.

Sharding: core c = (batch b = c//2, row-half = c%2). Each core computes one
batch's 64 output rows for all 384 out channels (3 M=128 chunks).

Per output tile [128 cout, 4 rows x 128 cols = 512] we accumulate 18 K=128
bf16 matmuls into one PSUM bank:
  - 12 taps x channel-chunk ci[0:128]  (from tile xa)
  - 5 tap-PAIRS x ci[128:192]          (from tile xb: lower 64 partitions =
    ci[128:192] data, upper 64 = same data shifted 1 col, so one K=128
    matmul covers two taps that differ by (0,+1))
  - 1 tap-pair (0,4)+(1,4) x ci[128:192] (tile xc: upper shifted one row)
Epilogue: one DVE scalar_tensor_tensor: out = (psum + bias) * valid.
"""

import numpy as np
import ml_dtypes

import concourse.bass as bass
import concourse.tile as tile
from concourse import mybir
from concourse.bass_utils import run_bass_kernel_spmd

B, CIN, COUT, H, W = 4, 192, 384, 128, 128
KH = KW = 5
PAD = 2
NCORES = 8
HHALF = 64          # output rows per core
NROWS = HHALF + 2   # input rows staged per core (2 above)
WP = W + 4          # padded width
FLAT = NROWS * WP   # 66*132 = 8712
RB = 4              # output rows per block
NBLK = HHALF // RB  # 16 blocks
NFREE = RB * W      # 512 = one PSUM bank of fp32

# Active taps of the 'A' mask, (kh, kw)
TAPS = [(0, 0), (0, 1), (0, 2), (0, 3), (0, 4),
        (1, 0), (1, 1), (1, 2), (1, 3), (1, 4),
        (2, 0), (2, 1)]
# ci[128:192] handled as pairs packed into K=128 matmuls.
# slab xb (upper shifted +1 element = +1 col): pairs differing by (0,1)
PAIRS_XB = [((0, 0), (0, 1)), ((0, 2), (0, 3)),
            ((1, 0), (1, 1)), ((1, 2), (1, 3)), ((2, 0), (2, 1))]
# slab xc (upper shifted +132 elements = +1 row): the leftover pair
PAIR_XC = ((0, 4), (1, 4))

BF16 = ml_dtypes.bfloat16


def _build_program():
    """Raw Bass (no Tile): this walrus build rejects instructions carrying
    more than ~1 embedded sync wait, so all synchronization is standalone
    wait_ge instructions with four manually-managed semaphores."""
    nc = bass.Bass()
    bf = mybir.dt.bfloat16
    f32 = mybir.dt.float32

    xa_d = nc.dram_tensor("xa", [128, FLAT], bf, kind="ExternalInput")
    xb_d = nc.dram_tensor("xb", [128, FLAT], bf, kind="ExternalInput")
    xc_d = nc.dram_tensor("xc", [128, FLAT], bf, kind="ExternalInput")
    wt_d = nc.dram_tensor("wt", [128, 18 * COUT], bf, kind="ExternalInput")
    bv_d = nc.dram_tensor("bv", [1, COUT + HHALF * W], bf, kind="ExternalInput")
    vt_d = nc.dram_tensor("vt", [128, HHALF * W], f32, kind="ExternalInput")
    out_d = nc.dram_tensor("out", [128, 3, HHALF * W], bf, kind="ExternalOutput")

    NPS = 4  # psum banks in rotation

    with (
        nc.sbuf_tensor([128, FLAT], bf) as xa_t,
        nc.sbuf_tensor([128, FLAT], bf) as xb_t,
        nc.sbuf_tensor([128, FLAT], bf) as xc_t,
        nc.sbuf_tensor([128, 18 * COUT], bf) as wt_t,
        nc.sbuf_tensor([1, COUT + HHALF * W], bf) as bv_t,
        nc.sbuf_tensor([128, HHALF * W], f32) as vt_t,
        nc.sbuf_tensor([128, 3 * HHALF * W], bf) as st_t,
        nc.psum_tensor([128, NPS * NFREE], f32) as ps_t,
        nc.semaphore("dwt") as dwt,
        nc.semaphore("dxa") as dxa,
        nc.semaphore("dxb") as dxb,
        nc.semaphore("dxc") as dxc,
        nc.semaphore("dbv") as dbv,
        nc.semaphore("dvt") as dvt,
        nc.semaphore("pes") as pes,
        nc.semaphore("dve") as dve,
        nc.semaphore("dout") as dout,
        nc.Block() as block,
    ):
        xa_v = xa_t[:].rearrange("p (r c) -> p r c", c=WP)
        xb_v = xb_t[:].rearrange("p (r c) -> p r c", c=WP)
        xc_v = xc_t[:].rearrange("p (r c) -> p r c", c=WP)

        # slot list: (view, kh, kw, input-DMA sem gating that view)
        slots = [(xa_v, kh, kw, dxa) for (kh, kw) in TAPS]
        slots += [(xb_v, ta[0], ta[1], dxb) for (ta, _tb) in PAIRS_XB]
        slots += [(xc_v, PAIR_XC[0][0], PAIR_XC[0][1], dxc)]

        @block.sync
        def _(sync):
            sync.dma_start(wt_t[:], wt_d[:]).then_inc(dwt, 16)
            sync.dma_start(xa_t[:], xa_d[:]).then_inc(dxa, 16)
            sync.dma_start(bv_t[:], bv_d[:]).then_inc(dbv, 16)
            sync.dma_start(xb_t[:], xb_d[:]).then_inc(dxb, 16)
            sync.dma_start(xc_t[:], xc_d[:]).then_inc(dxc, 16)
            sync.dma_start(vt_t[:], vt_d[:]).then_inc(dvt, 16)
            for m in range(3):
                sync.wait_ge(dve, NBLK * (m + 1))
                sync.dma_start(
                    out_d[:, m, :],
                    st_t[:, m * HHALF * W:(m + 1) * HHALF * W],
                ).then_inc(dout, 16)
            sync.wait_ge(dout, 48)

        @block.tensor
        def _(tensor):
            seen = set()

            def gate(sem):
                if sem.name not in seen:
                    seen.add(sem.name)
                    tensor.wait_ge(sem, 16)

            for k in range(3 * NBLK):  # tile index: m-major, blk minor
                m, blk = divmod(k, NBLK)
                j0 = blk * RB
                ps = ps_t[:, (k % NPS) * NFREE:(k % NPS + 1) * NFREE]
                if k >= NPS:
                    # bank reuse: DVE must have consumed tile k-NPS
                    tensor.wait_ge(dve, k - NPS + 1)
                for s, (view, kh, kw, sem) in enumerate(slots):
                    if k == 0:
                        if s == 0:
                            gate(dwt)
                        gate(sem)
                    nc.tensor.matmul(
                        ps,
                        wt_t[:, s * COUT + m * 128: s * COUT + (m + 1) * 128],
                        view[:, j0 + kh: j0 + kh + RB, kw: kw + W],
                        start=(s == 0),
                        stop=False,
                    )
                # bias*valid as a rank-1 accumulate; valid is 0/1 so the
                # final multiply by valid keeps (conv + bias) * valid exact.
                if k == 0:
                    gate(dbv)
                nc.tensor.matmul(
                    ps,
                    bv_t[0:1, m * 128:(m + 1) * 128],
                    bv_t[0:1, COUT + blk * NFREE:COUT + (blk + 1) * NFREE],
                    start=False,
                    stop=True,
                ).then_inc(pes, 1)

        @block.vector
        def _(vector):
            vector.wait_ge(dvt, 16)  # vt resident
            for k in range(3 * NBLK):
                m, blk = divmod(k, NBLK)
                ps = ps_t[:, (k % NPS) * NFREE:(k % NPS + 1) * NFREE]
                vector.wait_ge(pes, k + 1)
                nc.vector.tensor_mul(
                    st_t[:, (m * NBLK + blk) * NFREE:(m * NBLK + blk + 1) * NFREE],
                    ps,
                    vt_t[:, blk * NFREE:(blk + 1) * NFREE],
                ).then_inc(dve, 1)
    return nc


def _causal_mask():
    m = np.ones((KH, KW), dtype=np.float32)
    m[KH // 2, KW // 2:] = 0.0
    m[KH // 2 + 1:, :] = 0.0
    return m


def _prepare_in_maps(x, weight, bias, mask):
    # window-any of mask -> valid [B, H, W] float32
    ind = (np.asarray(mask)[:, 0] != 0)
    indp = np.zeros((B, H + 2 * PAD, W + 2 * PAD), dtype=bool)
    indp[:, PAD:PAD + H, PAD:PAD + W] = ind
    valid = np.zeros((B, H, W), dtype=bool)
    for dh in range(KH):
        for dw in range(KW):
            valid |= indp[:, dh:dh + H, dw:dw + W]
    valid_f = valid.astype(np.float32)

    w_bf = (np.asarray(weight, dtype=np.float32) * _causal_mask()[None, None]).astype(BF16)

    # 18 weight tiles [K=128, M=384] -> one SBUF image [128, 18, 384]
    wt = np.zeros((18, 128, COUT), dtype=BF16)
    for s, (kh, kw) in enumerate(TAPS):
        wt[s] = w_bf[:, 0:128, kh, kw].T
    for i, (ta, tb) in enumerate(PAIRS_XB):
        wt[12 + i, 0:64] = w_bf[:, 128:192, ta[0], ta[1]].T
        wt[12 + i, 64:128] = w_bf[:, 128:192, tb[0], tb[1]].T
    ta, tb = PAIR_XC
    wt[17, 0:64] = w_bf[:, 128:192, ta[0], ta[1]].T
    wt[17, 64:128] = w_bf[:, 128:192, tb[0], tb[1]].T
    wt_sb = np.ascontiguousarray(wt.transpose(1, 0, 2))

    bias_b = np.asarray(bias, dtype=np.float32).astype(BF16).reshape(1, COUT)

    x_bf = np.asarray(x, dtype=np.float32).astype(BF16)

    in_maps = []
    for c in range(NCORES):
        b, half = c // 2, c % 2
        r0 = half * HHALF
        xp = np.zeros((CIN, NROWS, WP), dtype=BF16)
        lo = r0 - PAD
        src_lo = max(lo, 0)
        xp[:, src_lo - lo:, PAD:PAD + W] = x_bf[b, :, src_lo:r0 + HHALF, :]
        xf = xp.reshape(CIN, FLAT)
        x2 = xf[128:192]
        sh1 = np.zeros_like(x2)
        sh1[:, :-1] = x2[:, 1:]
        shr = np.zeros_like(x2)
        shr[:, :-WP] = x2[:, WP:]
        vrow = valid_f[b, r0:r0 + HHALF].reshape(1, HHALF * W)
        vt = np.ascontiguousarray(np.broadcast_to(vrow, (128, HHALF * W)))
        in_maps.append({
            "xa": np.ascontiguousarray(xf[0:128]),
            "xb": np.ascontiguousarray(np.concatenate([x2, sh1], axis=0)),
            "xc": np.ascontiguousarray(np.concatenate([x2, shr], axis=0)),
            "wt": wt_sb.reshape(128, 18 * COUT),
            "bv": np.concatenate([bias_b, vrow.astype(BF16)], axis=1),
            "vt": vt,
        })
    return in_maps


def _assemble(results):
    out_full = np.zeros((B, COUT, H, W), dtype=np.float32)
    for c in range(NCORES):
        b, half = c // 2, c % 2
        o = np.asarray(results[c]["out"]).astype(np.float32)
        o4 = o.reshape(128, 3, HHALF, W).transpose(1, 0, 2, 3).reshape(COUT, HHALF, W)
        out_full[b, :, half * HHALF:(half + 1) * HHALF, :] = o4
    return out_full


def kernel(x, weight, bias, mask, _trace=False):
    in_maps = _prepare_in_maps(x, weight, bias, mask)
    nc = _build_program()
    res = run_bass_kernel_spmd(nc, in_maps, core_ids=list(range(NCORES)),
                               trace=_trace)
    out = _assemble(res.results)
    if _trace:
        return out, res
    return out


# revision 29
# speedup vs baseline: 1.1442x; 1.1442x over previous
"""Masked 5x5 conv (PixelCNN 'A' mask) on 8 Trainium2 NeuronCores.

Problem (hardcoded): x[4,192,128,128] f32, weight[384,192,5,5] f32,
bias[384] f32, mask[4,1,128,128] i32.
out = where(window_any(mask), conv(x, weight*maskA) + bias, 0).

The 'A' causal mask keeps 12 of 25 taps: rows kh=0,1 fully, row kh=2 only
kw=0,1 -- i.e. every tap reads the current output row or rows above it.

Sharding: core c = (batch b = c//2, row-half = c%2). Each core computes one
batch's 64 output rows for all 384 out channels (3 M=128 chunks).

Per output tile [128 cout, 4 rows x 128 cols = 512] we accumulate 18 K=128
bf16 matmuls into one PSUM bank:
  - 12 taps x channel-chunk ci[0:128]  (from tile xa)
  - 5 tap-PAIRS x ci[128:192]          (from tile xb: lower 64 partitions =
    ci[128:192] data, upper 64 = same data shifted 1 col, so one K=128
    matmul covers two taps that differ by (0,+1))
  - 1 tap-pair (0,4)+(1,4) x ci[128:192] (tile xc: upper shifted one row)
Epilogue: one DVE scalar_tensor_tensor: out = (psum + bias) * valid.
"""

import numpy as np
import ml_dtypes

import concourse.bass as bass
import concourse.tile as tile
from concourse import mybir
from concourse.bass_utils import run_bass_kernel_spmd

B, CIN, COUT, H, W = 4, 192, 384, 128, 128
KH = KW = 5
PAD = 2
NCORES = 8
HHALF = 64          # output rows per core
NROWS = HHALF + 2   # input rows staged per core (2 above)
WP = W + 4          # padded width
FLAT = NROWS * WP   # 66*132 = 8712
RB = 4              # output rows per block
NBLK = HHALF // RB  # 16 blocks
NFREE = RB * W      # 512 = one PSUM bank of fp32

# Active taps of the 'A' mask, (kh, kw)
TAPS = [(0, 0), (0, 1), (0, 2), (0, 3), (0, 4),
        (1, 0), (1, 1), (1, 2), (1, 3), (1, 4),
        (2, 0), (2, 1)]
# ci[128:192] handled as pairs packed into K=128 matmuls.
# slab xb (upper shifted +1 element = +1 col): pairs differing by (0,1)
PAIRS_XB = [((0, 0), (0, 1)), ((0, 2), (0, 3)),
            ((1, 0), (1, 1)), ((1, 2), (1, 3)), ((2, 0), (2, 1))]
# slab xc (upper shifted +132 elements = +1 row): the leftover pair
PAIR_XC = ((0, 4), (1, 4))

BF16 = ml_dtypes.bfloat16


def _build_program():
    """Raw Bass (no Tile): this walrus build rejects instructions carrying
    more than ~1 embedded sync wait, so all synchronization is standalone
    wait_ge instructions with four manually-managed semaphores."""
    nc = bass.Bass()
    bf = mybir.dt.bfloat16
    f32 = mybir.dt.float32

    xa_d = nc.dram_tensor("xa", [128, FLAT], bf, kind="ExternalInput")
    xb_d = nc.dram_tensor("xb", [128, FLAT], bf, kind="ExternalInput")
    xc_d = nc.dram_tensor("xc", [128, FLAT], bf, kind="ExternalInput")
    wt_d = nc.dram_tensor("wt", [128, 18 * COUT], bf, kind="ExternalInput")
    bt_d = nc.dram_tensor("bt", [128, 3], f32, kind="ExternalInput")
    vt_d = nc.dram_tensor("vt", [128, HHALF * W], f32, kind="ExternalInput")
    out_d = nc.dram_tensor("out", [128, 3 * HHALF * W], bf, kind="ExternalOutput")

    NPS = 8           # psum banks in rotation
    PHA = 8           # tiles 0..PHA-1 run split-phase (xa first, xb/xc later)
    XA1 = 38 * WP     # xa chunk 1 covers input rows 0..37 (output blocks 0..7)
    OCH = 8           # out-DMA granularity: blocks per chunk
    NT = 3 * NBLK     # 48 tiles

    with (
        nc.sbuf_tensor([128, FLAT], bf) as xa_t,
        nc.sbuf_tensor([128, FLAT], bf) as xb_t,
        nc.sbuf_tensor([128, FLAT], bf) as xc_t,
        nc.sbuf_tensor([128, 18 * COUT], bf) as wt_t,
        nc.sbuf_tensor([128, 3], f32) as bt_t,
        nc.sbuf_tensor([128, HHALF * W], f32) as vt_t,
        nc.sbuf_tensor([128, 3 * HHALF * W], bf) as st_t,
        nc.psum_tensor([128, NPS * NFREE], f32) as ps_t,
        nc.semaphore("dxa") as dxa,
        nc.semaphore("dxb") as dxb,
        nc.semaphore("dxc") as dxc,
        nc.semaphore("dwt") as dwt,
        nc.semaphore("drest") as drest,
        nc.semaphore("pes") as pes,
        nc.semaphore("dve") as dve,
        nc.semaphore("dout") as dout,
        nc.Block() as block,
    ):
        xa_v = xa_t[:].rearrange("p (r c) -> p r c", c=WP)
        xb_v = xb_t[:].rearrange("p (r c) -> p r c", c=WP)
        xc_v = xc_t[:].rearrange("p (r c) -> p r c", c=WP)

        # (global weight-slot index, view, kh, kw, gating sem)
        slots_a = [(s, xa_v, kh, kw, None) for s, (kh, kw) in enumerate(TAPS)]
        slots_bc = [(12 + i, xb_v, ta[0], ta[1], dxb)
                    for i, (ta, _tb) in enumerate(PAIRS_XB)]
        slots_bc += [(17, xc_v, PAIR_XC[0][0], PAIR_XC[0][1], dxc)]

        def emit_mms(tensor, k, sl, start, stop, gates):
            m, blk = divmod(k, NBLK)
            j0 = blk * RB
            ps = ps_t[:, (k % NPS) * NFREE:(k % NPS + 1) * NFREE]
            n = len(sl)
            for i, (s, view, kh, kw, sem) in enumerate(sl):
                if sem is not None and sem.name not in gates:
                    gates.add(sem.name)
                    tensor.wait_ge(sem, 16)
                mm = nc.tensor.matmul(
                    ps,
                    wt_t[:, s * COUT + m * 128: s * COUT + (m + 1) * 128],
                    view[:, j0 + kh: j0 + kh + RB, kw: kw + W],
                    start=(start and i == 0),
                    stop=(stop and i == n - 1),
                )
                if stop and i == n - 1:
                    mm.then_inc(pes, 1)

        @block.sync
        def _(sync):
            # serialized chain so early data gets full bandwidth
            sync.dma_start(wt_t[:], wt_d[:]).then_inc(dwt, 16)
            sync.dma_start(xa_t[:, 0:XA1], xa_d[:, 0:XA1]).then_inc(dxa, 16)
            sync.wait_ge(dxa, 16)
            sync.dma_start(xa_t[:, XA1:], xa_d[:, XA1:]).then_inc(dxa, 16)
            sync.wait_ge(dxa, 32)
            sync.dma_start(xb_t[:], xb_d[:]).then_inc(dxb, 16)
            sync.wait_ge(dxb, 16)
            sync.dma_start(xc_t[:], xc_d[:]).then_inc(dxc, 16)
            sync.wait_ge(dxc, 16)
            sync.dma_start(bt_t[:], bt_d[:]).then_inc(drest, 16)
            sync.dma_start(vt_t[:], vt_d[:]).then_inc(drest, 16)
            nch = NT // OCH
            for c in range(nch):
                sync.wait_ge(dve, OCH * (c + 1))
                sync.dma_start(
                    out_d[:, c * OCH * NFREE:(c + 1) * OCH * NFREE],
                    st_t[:, c * OCH * NFREE:(c + 1) * OCH * NFREE],
                ).then_inc(dout, 16)
            sync.wait_ge(dout, 16 * nch)

        @block.tensor
        def _(tensor):
            gates = set()
            # phase A: xa-only accumulation for tiles 0..PHA-1 -- runs as
            # soon as wt + first xa chunk land, hiding the xb/xc stream
            tensor.wait_ge(dwt, 16)
            tensor.wait_ge(dxa, 16)
            for k in range(PHA):
                emit_mms(tensor, k, slots_a, start=True, stop=False, gates=gates)
            # phase B: finish tiles 0..PHA-1 with the xb/xc pair slots
            for k in range(PHA):
                emit_mms(tensor, k, slots_bc, start=False, stop=True, gates=gates)
            # steady state
            gates.add(dxa.name)
            tensor.wait_ge(dxa, 32)
            for k in range(PHA, NT):
                tensor.wait_ge(dve, k - NPS + 1)
                emit_mms(tensor, k, slots_a, start=True, stop=False, gates=gates)
                emit_mms(tensor, k, slots_bc, start=False, stop=True, gates=gates)

        @block.vector
        def _(vector):
            vector.wait_ge(drest, 32)  # bias + valid resident
            for k in range(NT):
                m, blk = divmod(k, NBLK)
                ps = ps_t[:, (k % NPS) * NFREE:(k % NPS + 1) * NFREE]
                vector.wait_ge(pes, k + 1)
                nc.vector.scalar_tensor_tensor(
                    st_t[:, k * NFREE:(k + 1) * NFREE],
                    ps,
                    bt_t[:, m:m + 1],
                    vt_t[:, blk * NFREE:(blk + 1) * NFREE],
                    mybir.AluOpType.add,
                    mybir.AluOpType.mult,
                ).then_inc(dve, 1)
    return nc


def _causal_mask():
    m = np.ones((KH, KW), dtype=np.float32)
    m[KH // 2, KW // 2:] = 0.0
    m[KH // 2 + 1:, :] = 0.0
    return m


def _prepare_in_maps(x, weight, bias, mask):
    # window-any of mask -> valid [B, H, W] float32
    ind = (np.asarray(mask)[:, 0] != 0)
    indp = np.zeros((B, H + 2 * PAD, W + 2 * PAD), dtype=bool)
    indp[:, PAD:PAD + H, PAD:PAD + W] = ind
    valid = np.zeros((B, H, W), dtype=bool)
    for dh in range(KH):
        for dw in range(KW):
            valid |= indp[:, dh:dh + H, dw:dw + W]
    valid_f = valid.astype(np.float32)

    w_bf = (np.asarray(weight, dtype=np.float32) * _causal_mask()[None, None]).astype(BF16)

    # 18 weight tiles [K=128, M=384] -> one SBUF image [128, 18, 384]
    wt = np.zeros((18, 128, COUT), dtype=BF16)
    for s, (kh, kw) in enumerate(TAPS):
        wt[s] = w_bf[:, 0:128, kh, kw].T
    for i, (ta, tb) in enumerate(PAIRS_XB):
        wt[12 + i, 0:64] = w_bf[:, 128:192, ta[0], ta[1]].T
        wt[12 + i, 64:128] = w_bf[:, 128:192, tb[0], tb[1]].T
    ta, tb = PAIR_XC
    wt[17, 0:64] = w_bf[:, 128:192, ta[0], ta[1]].T
    wt[17, 64:128] = w_bf[:, 128:192, tb[0], tb[1]].T
    wt_sb = np.ascontiguousarray(wt.transpose(1, 0, 2))

    bias_t = np.ascontiguousarray(
        np.asarray(bias, dtype=np.float32).reshape(3, 128).T)

    x_bf = np.asarray(x, dtype=np.float32).astype(BF16)

    in_maps = []
    for c in range(NCORES):
        b, half = c // 2, c % 2
        r0 = half * HHALF
        xp = np.zeros((CIN, NROWS, WP), dtype=BF16)
        lo = r0 - PAD
        src_lo = max(lo, 0)
        xp[:, src_lo - lo:, PAD:PAD + W] = x_bf[b, :, src_lo:r0 + HHALF, :]
        xf = xp.reshape(CIN, FLAT)
        x2 = xf[128:192]
        sh1 = np.zeros_like(x2)
        sh1[:, :-1] = x2[:, 1:]
        shr = np.zeros_like(x2)
        shr[:, :-WP] = x2[:, WP:]
        vrow = valid_f[b, r0:r0 + HHALF].reshape(1, HHALF * W)
        vt = np.ascontiguousarray(np.broadcast_to(vrow, (128, HHALF * W)))
        in_maps.append({
            "xa": np.ascontiguousarray(xf[0:128]),
            "xb": np.ascontiguousarray(np.concatenate([x2, sh1], axis=0)),
            "xc": np.ascontiguousarray(np.concatenate([x2, shr], axis=0)),
            "wt": wt_sb.reshape(128, 18 * COUT),
            "bt": bias_t,
            "vt": vt,
        })
    return in_maps


def _assemble(results):
    out_full = np.zeros((B, COUT, H, W), dtype=np.float32)
    for c in range(NCORES):
        b, half = c // 2, c % 2
        o = np.asarray(results[c]["out"]).astype(np.float32)
        o4 = o.reshape(128, 3, HHALF, W).transpose(1, 0, 2, 3).reshape(COUT, HHALF, W)
        out_full[b, :, half * HHALF:(half + 1) * HHALF, :] = o4
    return out_full


def kernel(x, weight, bias, mask, _trace=False):
    in_maps = _prepare_in_maps(x, weight, bias, mask)
    nc = _build_program()
    res = run_bass_kernel_spmd(nc, in_maps, core_ids=list(range(NCORES)),
                               trace=_trace)
    out = _assemble(res.results)
    if _trace:
        return out, res
    return out


# revision 36
# speedup vs baseline: 1.1673x; 1.0202x over previous
"""Masked 5x5 conv (PixelCNN 'A' mask) on 8 Trainium2 NeuronCores.

Problem (hardcoded): x[4,192,128,128] f32, weight[384,192,5,5] f32,
bias[384] f32, mask[4,1,128,128] i32.
out = where(window_any(mask), conv(x, weight*maskA) + bias, 0).

The 'A' causal mask keeps 12 of 25 taps: rows kh=0,1 fully, row kh=2 only
kw=0,1 -- i.e. every tap reads the current output row or rows above it.

Sharding: core c = (batch b = c//2, row-half = c%2). Each core computes one
batch's 64 output rows for all 384 out channels (3 M=128 chunks).

Per output tile [128 cout, 4 rows x 128 cols = 512] we accumulate 18 K=128
bf16 matmuls into one PSUM bank:
  - 12 taps x channel-chunk ci[0:128]  (from tile xa)
  - 5 tap-PAIRS x ci[128:192]          (from tile xb: lower 64 partitions =
    ci[128:192] data, upper 64 = same data shifted 1 col, so one K=128
    matmul covers two taps that differ by (0,+1))
  - 1 tap-pair (0,4)+(1,4) x ci[128:192] (tile xc: upper shifted one row)
Epilogue: one DVE scalar_tensor_tensor: out = (psum + bias) * valid.
"""

import numpy as np
import ml_dtypes

import concourse.bass as bass
import concourse.tile as tile
from concourse import mybir
from concourse.bass_utils import run_bass_kernel_spmd

B, CIN, COUT, H, W = 4, 192, 384, 128, 128
KH = KW = 5
PAD = 2
NCORES = 8
HHALF = 64          # output rows per core
NROWS = HHALF + 2   # input rows staged per core (2 above)
WP = W + 4          # padded width
FLAT = NROWS * WP   # 66*132 = 8712
RB = 4              # output rows per block
NBLK = HHALF // RB  # 16 blocks
NFREE = RB * W      # 512 = one PSUM bank of fp32

# Active taps of the 'A' mask, (kh, kw)
TAPS = [(0, 0), (0, 1), (0, 2), (0, 3), (0, 4),
        (1, 0), (1, 1), (1, 2), (1, 3), (1, 4),
        (2, 0), (2, 1)]
# ci[128:192] handled as pairs packed into K=128 matmuls.
# slab xb (upper shifted +1 element = +1 col): pairs differing by (0,1)
PAIRS_XB = [((0, 0), (0, 1)), ((0, 2), (0, 3)),
            ((1, 0), (1, 1)), ((1, 2), (1, 3)), ((2, 0), (2, 1))]
# slab xc (upper shifted +132 elements = +1 row): the leftover pair
PAIR_XC = ((0, 4), (1, 4))

BF16 = ml_dtypes.bfloat16


def _build_program():
    """Raw Bass (no Tile): this walrus build rejects instructions carrying
    more than ~1 embedded sync wait, so all synchronization is standalone
    wait_ge instructions with four manually-managed semaphores."""
    nc = bass.Bass()
    bf = mybir.dt.bfloat16
    f32 = mybir.dt.float32

    xa_d = nc.dram_tensor("xa", [128, FLAT], bf, kind="ExternalInput")
    xb_d = nc.dram_tensor("xb", [128, FLAT], bf, kind="ExternalInput")
    xc_d = nc.dram_tensor("xc", [128, FLAT], bf, kind="ExternalInput")
    wt_d = nc.dram_tensor("wt", [128, 18 * COUT], bf, kind="ExternalInput")
    bt_d = nc.dram_tensor("bt", [128, 3], f32, kind="ExternalInput")
    vt_d = nc.dram_tensor("vt", [128, HHALF * W], bf, kind="ExternalInput")
    out_d = nc.dram_tensor("out", [128, 3 * HHALF * W], bf, kind="ExternalOutput")

    NPS = 8           # psum banks in rotation
    PHA = 8           # tiles 0..PHA-1 run split-phase (xa first, xb/xc later)
    XA1 = 38 * WP     # xa chunk 1 covers input rows 0..37 (output blocks 0..7)
    OCH = 4           # out-DMA granularity: blocks per chunk
    NT = 3 * NBLK     # 48 tiles

    with (
        nc.sbuf_tensor([128, FLAT], bf) as xa_t,
        nc.sbuf_tensor([128, FLAT], bf) as xb_t,
        nc.sbuf_tensor([128, FLAT], bf) as xc_t,
        nc.sbuf_tensor([128, 18 * COUT], bf) as wt_t,
        nc.sbuf_tensor([128, 3], f32) as bt_t,
        nc.sbuf_tensor([128, HHALF * W], bf) as vt_t,
        nc.sbuf_tensor([128, 3 * HHALF * W], bf) as st_t,
        nc.psum_tensor([128, NPS * NFREE], f32) as ps_t,
        nc.semaphore("dxa1") as dxa1,
        nc.semaphore("dxa2") as dxa2,
        nc.semaphore("dxb1") as dxb1,
        nc.semaphore("dxb2") as dxb2,
        nc.semaphore("dxc1") as dxc1,
        nc.semaphore("dxc2") as dxc2,
        nc.semaphore("dwt1") as dwt1,
        nc.semaphore("dwt2") as dwt2,
        nc.semaphore("drest") as drest,
        nc.semaphore("pes") as pes,
        nc.semaphore("dve") as dve,
        nc.semaphore("dout") as dout,
        nc.Block() as block,
    ):
        xa_v = xa_t[:].rearrange("p (r c) -> p r c", c=WP)
        xb_v = xb_t[:].rearrange("p (r c) -> p r c", c=WP)
        xc_v = xc_t[:].rearrange("p (r c) -> p r c", c=WP)

        # (global weight-slot index, view, kh, kw)
        slots_a = [(s, xa_v, kh, kw) for s, (kh, kw) in enumerate(TAPS)]
        slots_bc = [(12 + i, xb_v, ta[0], ta[1])
                    for i, (ta, _tb) in enumerate(PAIRS_XB)]
        slots_bc += [(17, xc_v, PAIR_XC[0][0], PAIR_XC[0][1])]

        def emit_mms(tensor, k, sl, start, stop):
            m, blk = divmod(k, NBLK)
            j0 = blk * RB
            ps = ps_t[:, (k % NPS) * NFREE:(k % NPS + 1) * NFREE]
            n = len(sl)
            for i, (s, view, kh, kw) in enumerate(sl):
                mm = nc.tensor.matmul(
                    ps,
                    wt_t[:, s * COUT + m * 128: s * COUT + (m + 1) * 128],
                    view[:, j0 + kh: j0 + kh + RB, kw: kw + W],
                    start=(start and i == 0),
                    stop=(stop and i == n - 1),
                )
                if stop and i == n - 1:
                    mm.then_inc(pes, 1)

        @block.sync
        def _(sync):
            # wave 1: what phase A needs -- wt slots 0..11 + xa rows 0..37,
            # split in halves so multiple HW queues stream in parallel
            WT1 = 12 * COUT  # wt cols for the 12 xa slots
            sync.dma_start(wt_t[:, 0:WT1 // 2], wt_d[:, 0:WT1 // 2]).then_inc(dwt1, 16)
            sync.dma_start(wt_t[:, WT1 // 2:WT1], wt_d[:, WT1 // 2:WT1]).then_inc(dwt1, 16)
            sync.dma_start(xa_t[:, 0:XA1 // 2], xa_d[:, 0:XA1 // 2]).then_inc(dxa1, 16)
            sync.dma_start(xa_t[:, XA1 // 2:XA1], xa_d[:, XA1 // 2:XA1]).then_inc(dxa1, 16)
            # wave 2: what phase B + the DVE epilogue need
            sync.dma_start(wt_t[:, WT1:], wt_d[:, WT1:]).then_inc(dwt2, 16)
            sync.dma_start(xb_t[:, 0:XA1], xb_d[:, 0:XA1]).then_inc(dxb1, 16)
            sync.dma_start(xc_t[:, 0:XA1], xc_d[:, 0:XA1]).then_inc(dxc1, 16)
            sync.dma_start(bt_t[:], bt_d[:]).then_inc(drest, 16)
            sync.dma_start(vt_t[:], vt_d[:]).then_inc(drest, 16)
            # wave 3: steady-state remainders
            sync.dma_start(xa_t[:, XA1:], xa_d[:, XA1:]).then_inc(dxa2, 16)
            sync.dma_start(xb_t[:, XA1:], xb_d[:, XA1:]).then_inc(dxb2, 16)
            sync.dma_start(xc_t[:, XA1:], xc_d[:, XA1:]).then_inc(dxc2, 16)
            nch = NT // OCH
            for c in range(nch):
                sync.wait_ge(dve, OCH * (c + 1))
                sync.dma_start(
                    out_d[:, c * OCH * NFREE:(c + 1) * OCH * NFREE],
                    st_t[:, c * OCH * NFREE:(c + 1) * OCH * NFREE],
                ).then_inc(dout, 16)
            sync.wait_ge(dout, 16 * nch)

        @block.tensor
        def _(tensor):
            # phase A: xa-only accumulation for tiles 0..PHA-1 -- runs as
            # soon as wt slots 0..11 + xa rows 0..37 land
            tensor.wait_ge(dwt1, 32)
            tensor.wait_ge(dxa1, 32)
            for k in range(PHA):
                emit_mms(tensor, k, slots_a, start=True, stop=False)
            # phase B: finish tiles 0..PHA-1 with the xb/xc pair slots
            tensor.wait_ge(dwt2, 16)
            tensor.wait_ge(dxb1, 16)
            tensor.wait_ge(dxc1, 16)
            for k in range(PHA):
                emit_mms(tensor, k, slots_bc, start=False, stop=True)
            # steady state
            tensor.wait_ge(dxa2, 16)
            tensor.wait_ge(dxb2, 16)
            tensor.wait_ge(dxc2, 16)
            for k in range(PHA, NT):
                tensor.wait_ge(dve, k - NPS + 1)
                emit_mms(tensor, k, slots_a, start=True, stop=False)
                emit_mms(tensor, k, slots_bc, start=False, stop=True)

        @block.vector
        def _(vector):
            vector.wait_ge(drest, 32)  # bias + valid resident
            for k in range(NT):
                m, blk = divmod(k, NBLK)
                ps = ps_t[:, (k % NPS) * NFREE:(k % NPS + 1) * NFREE]
                vector.wait_ge(pes, k + 1)
                nc.vector.scalar_tensor_tensor(
                    st_t[:, k * NFREE:(k + 1) * NFREE],
                    ps,
                    bt_t[:, m:m + 1],
                    vt_t[:, blk * NFREE:(blk + 1) * NFREE],
                    mybir.AluOpType.add,
                    mybir.AluOpType.mult,
                ).then_inc(dve, 1)
    return nc


def _causal_mask():
    m = np.ones((KH, KW), dtype=np.float32)
    m[KH // 2, KW // 2:] = 0.0
    m[KH // 2 + 1:, :] = 0.0
    return m


def _prepare_in_maps(x, weight, bias, mask):
    # window-any of mask -> valid [B, H, W] float32
    ind = (np.asarray(mask)[:, 0] != 0)
    indp = np.zeros((B, H + 2 * PAD, W + 2 * PAD), dtype=bool)
    indp[:, PAD:PAD + H, PAD:PAD + W] = ind
    valid = np.zeros((B, H, W), dtype=bool)
    for dh in range(KH):
        for dw in range(KW):
            valid |= indp[:, dh:dh + H, dw:dw + W]
    valid_f = valid.astype(np.float32)

    w_bf = (np.asarray(weight, dtype=np.float32) * _causal_mask()[None, None]).astype(BF16)

    # 18 weight tiles [K=128, M=384] -> one SBUF image [128, 18, 384]
    wt = np.zeros((18, 128, COUT), dtype=BF16)
    for s, (kh, kw) in enumerate(TAPS):
        wt[s] = w_bf[:, 0:128, kh, kw].T
    for i, (ta, tb) in enumerate(PAIRS_XB):
        wt[12 + i, 0:64] = w_bf[:, 128:192, ta[0], ta[1]].T
        wt[12 + i, 64:128] = w_bf[:, 128:192, tb[0], tb[1]].T
    ta, tb = PAIR_XC
    wt[17, 0:64] = w_bf[:, 128:192, ta[0], ta[1]].T
    wt[17, 64:128] = w_bf[:, 128:192, tb[0], tb[1]].T
    wt_sb = np.ascontiguousarray(wt.transpose(1, 0, 2))

    bias_t = np.ascontiguousarray(
        np.asarray(bias, dtype=np.float32).reshape(3, 128).T)

    x_bf = np.asarray(x, dtype=np.float32).astype(BF16)

    in_maps = []
    for c in range(NCORES):
        b, half = c // 2, c % 2
        r0 = half * HHALF
        xp = np.zeros((CIN, NROWS, WP), dtype=BF16)
        lo = r0 - PAD
        src_lo = max(lo, 0)
        xp[:, src_lo - lo:, PAD:PAD + W] = x_bf[b, :, src_lo:r0 + HHALF, :]
        xf = xp.reshape(CIN, FLAT)
        x2 = xf[128:192]
        sh1 = np.zeros_like(x2)
        sh1[:, :-1] = x2[:, 1:]
        shr = np.zeros_like(x2)
        shr[:, :-WP] = x2[:, WP:]
        vrow = valid_f[b, r0:r0 + HHALF].reshape(1, HHALF * W).astype(BF16)
        vt = np.ascontiguousarray(np.broadcast_to(vrow, (128, HHALF * W)))
        in_maps.append({
            "xa": np.ascontiguousarray(xf[0:128]),
            "xb": np.ascontiguousarray(np.concatenate([x2, sh1], axis=0)),
            "xc": np.ascontiguousarray(np.concatenate([x2, shr], axis=0)),
            "wt": wt_sb.reshape(128, 18 * COUT),
            "bt": bias_t,
            "vt": vt,
        })
    return in_maps


def _assemble(results):
    out_full = np.zeros((B, COUT, H, W), dtype=np.float32)
    for c in range(NCORES):
        b, half = c // 2, c % 2
        o = np.asarray(results[c]["out"]).astype(np.float32)
        o4 = o.reshape(128, 3, HHALF, W).transpose(1, 0, 2, 3).reshape(COUT, HHALF, W)
        out_full[b, :, half * HHALF:(half + 1) * HHALF, :] = o4
    return out_full


def kernel(x, weight, bias, mask, _trace=False):
    in_maps = _prepare_in_maps(x, weight, bias, mask)
    nc = _build_program()
    res = run_bass_kernel_spmd(nc, in_maps, core_ids=list(range(NCORES)),
                               trace=_trace)
    out = _assemble(res.results)
    if _trace:
        return out, res
    return out


# revision 41
# speedup vs baseline: 1.2003x; 1.0283x over previous
"""Masked 5x5 conv (PixelCNN 'A' mask) on 8 Trainium2 NeuronCores.

Problem (hardcoded): x[4,192,128,128] f32, weight[384,192,5,5] f32,
bias[384] f32, mask[4,1,128,128] i32.
out = where(window_any(mask), conv(x, weight*maskA) + bias, 0).

The 'A' causal mask keeps 12 of 25 taps: rows kh=0,1 fully, row kh=2 only
kw=0,1 -- i.e. every tap reads the current output row or rows above it.

Sharding: core c = (batch b = c//2, row-half = c%2). Each core computes one
batch's 64 output rows for all 384 out channels (3 M=128 chunks).

Per output tile [128 cout, 4 rows x 128 cols = 512] we accumulate 18 K=128
bf16 matmuls into one PSUM bank:
  - 12 taps x channel-chunk ci[0:128]  (from tile xa)
  - 5 tap-PAIRS x ci[128:192]          (from tile xb: lower 64 partitions =
    ci[128:192] data, upper 64 = same data shifted 1 col, so one K=128
    matmul covers two taps that differ by (0,+1))
  - 1 tap-pair (0,4)+(1,4) x ci[128:192] (tile xc: upper shifted one row)
Epilogue: one DVE scalar_tensor_tensor: out = (psum + bias) * valid.
"""

import numpy as np
import ml_dtypes

import concourse.bass as bass
import concourse.tile as tile
from concourse import mybir
from concourse.bass_utils import run_bass_kernel_spmd

B, CIN, COUT, H, W = 4, 192, 384, 128, 128
KH = KW = 5
PAD = 2
NCORES = 8
HHALF = 64          # output rows per core
NROWS = HHALF + 2   # input rows staged per core (2 above)
WP = W + 4          # padded width
FLAT = NROWS * WP   # 66*132 = 8712
RB = 4              # output rows per block
NBLK = HHALF // RB  # 16 blocks
NFREE = RB * W      # 512 = one PSUM bank of fp32

# Active taps of the 'A' mask, (kh, kw)
TAPS = [(0, 0), (0, 1), (0, 2), (0, 3), (0, 4),
        (1, 0), (1, 1), (1, 2), (1, 3), (1, 4),
        (2, 0), (2, 1)]
# ci[128:192] handled as pairs packed into K=128 matmuls.
# slab xb (upper shifted +1 element = +1 col): pairs differing by (0,1)
PAIRS_XB = [((0, 0), (0, 1)), ((0, 2), (0, 3)),
            ((1, 0), (1, 1)), ((1, 2), (1, 3)), ((2, 0), (2, 1))]
# slab xc (upper shifted +132 elements = +1 row): the leftover pair
PAIR_XC = ((0, 4), (1, 4))

BF16 = ml_dtypes.bfloat16


def _build_program():
    """Raw Bass (no Tile): this walrus build rejects instructions carrying
    more than ~1 embedded sync wait, so all synchronization is standalone
    wait_ge instructions with four manually-managed semaphores."""
    nc = bass.Bass()
    bf = mybir.dt.bfloat16
    f32 = mybir.dt.float32

    xa_d = nc.dram_tensor("xa", [128, FLAT], bf, kind="ExternalInput")
    xb_d = nc.dram_tensor("xb", [128, FLAT], bf, kind="ExternalInput")
    xc_d = nc.dram_tensor("xc", [128, FLAT], bf, kind="ExternalInput")
    wt_d = nc.dram_tensor("wt", [128, 18 * COUT], bf, kind="ExternalInput")
    bt_d = nc.dram_tensor("bt", [128, 3], f32, kind="ExternalInput")
    vt_d = nc.dram_tensor("vt", [128, HHALF * W], bf, kind="ExternalInput")
    out_d = nc.dram_tensor("out", [128, 3 * HHALF * W], bf, kind="ExternalOutput")

    NPS = 8           # psum banks in rotation
    PHA = 8           # tiles 0..PHA-1 run split-phase (xa first, xb/xc later)
    XA1 = 38 * WP     # xa chunk 1 covers input rows 0..37 (output blocks 0..7)
    OCH = 2           # out-DMA granularity: blocks per chunk
    NT = 3 * NBLK     # 48 tiles

    from contextlib import ExitStack
    with ExitStack() as ctx:
        xa_t = ctx.enter_context(nc.sbuf_tensor([128, FLAT], bf))
        xb_t = ctx.enter_context(nc.sbuf_tensor([128, FLAT], bf))
        xc_t = ctx.enter_context(nc.sbuf_tensor([128, FLAT], bf))
        wt_t = ctx.enter_context(nc.sbuf_tensor([128, 18 * COUT], bf))
        bt_t = ctx.enter_context(nc.sbuf_tensor([128, 3], f32))
        vt_t = ctx.enter_context(nc.sbuf_tensor([128, HHALF * W], bf))
        st_t = ctx.enter_context(nc.sbuf_tensor([128, 3 * HHALF * W], bf))
        ps_t = ctx.enter_context(nc.psum_tensor([128, NPS * NFREE], f32))
        da0 = ctx.enter_context(nc.semaphore("da0"))
        da1 = ctx.enter_context(nc.semaphore("da1"))
        da2 = ctx.enter_context(nc.semaphore("da2"))
        db1 = ctx.enter_context(nc.semaphore("db1"))
        db2 = ctx.enter_context(nc.semaphore("db2"))
        dc1 = ctx.enter_context(nc.semaphore("dc1"))
        dc2 = ctx.enter_context(nc.semaphore("dc2"))
        dwt1 = ctx.enter_context(nc.semaphore("dwt1"))
        dwt2 = ctx.enter_context(nc.semaphore("dwt2"))
        drest = ctx.enter_context(nc.semaphore("drest"))
        pes = ctx.enter_context(nc.semaphore("pes"))
        dve = ctx.enter_context(nc.semaphore("dve"))
        dout = ctx.enter_context(nc.semaphore("dout"))
        block = ctx.enter_context(nc.Block())
        xa_v = xa_t[:].rearrange("p (r c) -> p r c", c=WP)
        xb_v = xb_t[:].rearrange("p (r c) -> p r c", c=WP)
        xc_v = xc_t[:].rearrange("p (r c) -> p r c", c=WP)

        # (global weight-slot index, view, kh, kw)
        slots_a = [(s, xa_v, kh, kw) for s, (kh, kw) in enumerate(TAPS)]
        slots_bc = [(12 + i, xb_v, ta[0], ta[1])
                    for i, (ta, _tb) in enumerate(PAIRS_XB)]
        slots_bc += [(17, xc_v, PAIR_XC[0][0], PAIR_XC[0][1])]

        def emit_mms(tensor, k, sl, start, stop):
            m, blk = divmod(k, NBLK)
            j0 = blk * RB
            ps = ps_t[:, (k % NPS) * NFREE:(k % NPS + 1) * NFREE]
            n = len(sl)
            for i, (s, view, kh, kw) in enumerate(sl):
                mm = nc.tensor.matmul(
                    ps,
                    wt_t[:, s * COUT + m * 128: s * COUT + (m + 1) * 128],
                    view[:, j0 + kh: j0 + kh + RB, kw: kw + W],
                    start=(start and i == 0),
                    stop=(stop and i == n - 1),
                )
                if stop and i == n - 1:
                    mm.then_inc(pes, 1)

        @block.sync
        def _(sync):
            # DMA queues give ~45-90 GB/s per stream and ~358 GB/s aggregate,
            # so stream in prioritized serialized waves, each wave split
            # across a few queues. Wave 1a covers the very first matmuls.
            WT1 = 12 * COUT   # wt cols for the 12 xa slots
            XA0 = 14 * WP     # xa rows 0..13: blocks 0..2
            def split2(dst, src, lo, hi, sem):
                mid = ((lo + hi) // 2 // 4) * 4
                sync.dma_start(dst[:, lo:mid], src[:, lo:mid]).then_inc(sem, 16)
                sync.dma_start(dst[:, mid:hi], src[:, mid:hi]).then_inc(sem, 16)

            split2(wt_t, wt_d, 0, WT1, dwt1)
            split2(xa_t, xa_d, 0, XA0, da0)
            sync.wait_ge(dwt1, 32)
            sync.wait_ge(da0, 32)
            split2(xa_t, xa_d, XA0, XA1, da1)
            sync.wait_ge(da1, 32)
            # wave 2: phase-B inputs + DVE epilogue inputs; xb first
            split2(xb_t, xb_d, 0, XA1, db1)
            split2(xc_t, xc_d, 0, XA1, dc1)
            sync.dma_start(wt_t[:, WT1:], wt_d[:, WT1:]).then_inc(dwt2, 16)
            sync.dma_start(bt_t[:], bt_d[:]).then_inc(drest, 16)
            split2(vt_t, vt_d, 0, HHALF * W, drest)
            sync.wait_ge(db1, 32)
            sync.wait_ge(dc1, 32)
            # wave 3: steady-state remainders
            split2(xa_t, xa_d, XA1, FLAT, da2)
            split2(xb_t, xb_d, XA1, FLAT, db2)
            split2(xc_t, xc_d, XA1, FLAT, dc2)
            nch = NT // OCH
            for c in range(nch):
                sync.wait_ge(dve, OCH * (c + 1))
                sync.dma_start(
                    out_d[:, c * OCH * NFREE:(c + 1) * OCH * NFREE],
                    st_t[:, c * OCH * NFREE:(c + 1) * OCH * NFREE],
                ).then_inc(dout, 16)
            sync.wait_ge(dout, 16 * nch)

        @block.tensor
        def _(tensor):
            # phase A: xa-only accumulation for tiles 0..PHA-1, gated on the
            # just-in-time xa row chunks
            tensor.wait_ge(dwt1, 32)
            tensor.wait_ge(da0, 32)
            for k in range(3):
                emit_mms(tensor, k, slots_a, start=True, stop=False)
            tensor.wait_ge(da1, 32)
            for k in range(3, PHA):
                emit_mms(tensor, k, slots_a, start=True, stop=False)
            # phase B: finish tiles 0..PHA-1 with the xb/xc pair slots
            tensor.wait_ge(dwt2, 16)
            tensor.wait_ge(db1, 32)
            tensor.wait_ge(dc1, 32)
            for k in range(PHA):
                emit_mms(tensor, k, slots_bc, start=False, stop=True)
            # steady state
            tensor.wait_ge(da2, 32)
            tensor.wait_ge(db2, 32)
            tensor.wait_ge(dc2, 32)
            for k in range(PHA, NT):
                tensor.wait_ge(dve, k - NPS + 1)
                emit_mms(tensor, k, slots_a, start=True, stop=False)
                emit_mms(tensor, k, slots_bc, start=False, stop=True)

        @block.vector
        def _(vector):
            vector.wait_ge(drest, 48)  # bias + valid resident (3 chunks)
            for k in range(NT):
                m, blk = divmod(k, NBLK)
                ps = ps_t[:, (k % NPS) * NFREE:(k % NPS + 1) * NFREE]
                vector.wait_ge(pes, k + 1)
                nc.vector.scalar_tensor_tensor(
                    st_t[:, k * NFREE:(k + 1) * NFREE],
                    ps,
                    bt_t[:, m:m + 1],
                    vt_t[:, blk * NFREE:(blk + 1) * NFREE],
                    mybir.AluOpType.add,
                    mybir.AluOpType.mult,
                ).then_inc(dve, 1)
    return nc


def _causal_mask():
    m = np.ones((KH, KW), dtype=np.float32)
    m[KH // 2, KW // 2:] = 0.0
    m[KH // 2 + 1:, :] = 0.0
    return m


def _prepare_in_maps(x, weight, bias, mask):
    # window-any of mask -> valid [B, H, W] float32
    ind = (np.asarray(mask)[:, 0] != 0)
    indp = np.zeros((B, H + 2 * PAD, W + 2 * PAD), dtype=bool)
    indp[:, PAD:PAD + H, PAD:PAD + W] = ind
    valid = np.zeros((B, H, W), dtype=bool)
    for dh in range(KH):
        for dw in range(KW):
            valid |= indp[:, dh:dh + H, dw:dw + W]
    valid_f = valid.astype(np.float32)

    w_bf = (np.asarray(weight, dtype=np.float32) * _causal_mask()[None, None]).astype(BF16)

    # 18 weight tiles [K=128, M=384] -> one SBUF image [128, 18, 384]
    wt = np.zeros((18, 128, COUT), dtype=BF16)
    for s, (kh, kw) in enumerate(TAPS):
        wt[s] = w_bf[:, 0:128, kh, kw].T
    for i, (ta, tb) in enumerate(PAIRS_XB):
        wt[12 + i, 0:64] = w_bf[:, 128:192, ta[0], ta[1]].T
        wt[12 + i, 64:128] = w_bf[:, 128:192, tb[0], tb[1]].T
    ta, tb = PAIR_XC
    wt[17, 0:64] = w_bf[:, 128:192, ta[0], ta[1]].T
    wt[17, 64:128] = w_bf[:, 128:192, tb[0], tb[1]].T
    wt_sb = np.ascontiguousarray(wt.transpose(1, 0, 2))

    bias_t = np.ascontiguousarray(
        np.asarray(bias, dtype=np.float32).reshape(3, 128).T)

    x_bf = np.asarray(x, dtype=np.float32).astype(BF16)

    in_maps = []
    for c in range(NCORES):
        b, half = c // 2, c % 2
        r0 = half * HHALF
        xp = np.zeros((CIN, NROWS, WP), dtype=BF16)
        lo = r0 - PAD
        src_lo = max(lo, 0)
        xp[:, src_lo - lo:, PAD:PAD + W] = x_bf[b, :, src_lo:r0 + HHALF, :]
        xf = xp.reshape(CIN, FLAT)
        x2 = xf[128:192]
        sh1 = np.zeros_like(x2)
        sh1[:, :-1] = x2[:, 1:]
        shr = np.zeros_like(x2)
        shr[:, :-WP] = x2[:, WP:]
        vrow = valid_f[b, r0:r0 + HHALF].reshape(1, HHALF * W).astype(BF16)
        vt = np.ascontiguousarray(np.broadcast_to(vrow, (128, HHALF * W)))
        in_maps.append({
            "xa": np.ascontiguousarray(xf[0:128]),
            "xb": np.ascontiguousarray(np.concatenate([x2, sh1], axis=0)),
            "xc": np.ascontiguousarray(np.concatenate([x2, shr], axis=0)),
            "wt": wt_sb.reshape(128, 18 * COUT),
            "bt": bias_t,
            "vt": vt,
        })
    return in_maps


def _assemble(results):
    out_full = np.zeros((B, COUT, H, W), dtype=np.float32)
    for c in range(NCORES):
        b, half = c // 2, c % 2
        o = np.asarray(results[c]["out"]).astype(np.float32)
        o4 = o.reshape(128, 3, HHALF, W).transpose(1, 0, 2, 3).reshape(COUT, HHALF, W)
        out_full[b, :, half * HHALF:(half + 1) * HHALF, :] = o4
    return out_full


def kernel(x, weight, bias, mask, _trace=False):
    in_maps = _prepare_in_maps(x, weight, bias, mask)
    nc = _build_program()
    res = run_bass_kernel_spmd(nc, in_maps, core_ids=list(range(NCORES)),
                               trace=_trace)
    out = _assemble(res.results)
    if _trace:
        return out, res
    return out
